# revision 42
# baseline (speedup 1.0000x reference)
"""MoE layer (hash-routed, top-k=2, E=8 experts) on 8 Trainium2 NeuronCores.

Strategy: expert-parallel with QUARTER-LEVEL load balancing.  Each expert's
H=4096 hidden dim splits into 4 quarters of 1024; since ReLU is elementwise
in h, y_e = sum_j relu(x@W1[:, Qj] + b1[Qj]) @ W2[Qj, :] exactly.  That
gives 32 independent (expert, h-quarter) jobs, greedily packed 4-per-core
so every core's token total is ~mean expert load instead of the hottest
expert's load.  The host routes tokens, pre-tiles weights partition-major,
and sums the 4 partial outputs per expert (plus b2) during unscatter.

All matmul operands are bf16 (same 1-cycle/row PE rate as f32r, half the
traffic, LDWEIGHTS fully hidden); PSUM accumulates fp32; the partial y of
a job needs no cross-quarter SBUF accumulation, so layer 2 results go
PSUM -> bf16 -> HBM directly.

Per-core device kernel, jobs j=0..3 with per-slot token caps C_j:
  job j: layer1: H1T[h, tok] = relu(W1qj^T @ XTj + b1qj)
         layer2: YTj[d, tok] = W2qj^T @ H1T          (stored per job)
All loads ride one sync-HWDGE queue in consumption-priority order; 34
warm-up matmuls bridge the ~8us HBM fill phase so the PE clock ramp is
never reset by an idle gap.  Tokens are the moving/free dim in both
layers, so caps need no 128 padding.
"""

import numpy as np

import concourse.bass as bass
import concourse.mybir as mybir
import concourse.tile as tile
from concourse import bacc
from concourse.bass_utils import run_bass_kernel_spmd

dt = mybir.dt

B, S, D, H, E, NCORES = 4, 1024, 1024, 4096, 8, 8
HQ = 1024                      # h-quarter width
KT = D // 128                  # 8 contraction tiles (d)
DT = D // 128                  # 8 output d-tiles
HTQ = HQ // 128                # 8 h-tiles per quarter
NJ = H // HQ                   # 4 jobs per core (one h-quarter each)

MM_DT = "bf16"

_BUILD_CACHE: dict = {}


def _io_np_dtype():
    if MM_DT == "bf16":
        import ml_dtypes

        return np.dtype(ml_dtypes.bfloat16)
    return np.dtype(np.float32)


def _io_dt():
    return dt.bfloat16 if MM_DT == "bf16" else dt.float32


def _chunks(C):
    """Token chunks of <=512 (PSUM bank limit for fp32 out)."""
    out = []
    c0 = 0
    while c0 < C:
        n = min(512, C - c0)
        out.append((c0, n))
        c0 += n
    return out


def build_nc(caps: tuple):
    """Per-core program: NJ jobs with token caps caps[j]."""
    assert len(caps) == NJ
    for c in caps:
        assert c % 4 == 0 and 256 <= c <= 1280
    io_dt = _io_dt()
    y_dt = dt.bfloat16 if MM_DT == "bf16" else dt.float32

    nc = bacc.Bacc(
        "TRN2",
        target_bir_lowering=False,
        debug=False,
        num_devices=NCORES,
    )

    # per-job tensors, host pre-tiled partition-major:
    #   xtJ: [128, KT*Cj]     xt[p, kt*Cj+c] = x_tok[c, kt*128+p]
    #   w1J: [128, HTQ*KT*128] quarter slice of W1, tiled like xt
    #   w2J: [128, HTQ*D]      quarter rows of W2
    #   yJ:  [128, DT*Cj]      YT[d, tok] partial (bf16)
    xt_d, w1_d, w2_d, y_d = [], [], [], []
    for j in range(NJ):
        xt_d.append(nc.dram_tensor(f"xt{j}", [128, KT * caps[j]], io_dt,
                                   kind="ExternalInput"))
        w1_d.append(nc.dram_tensor(f"w1{j}", [128, HTQ * KT * 128], io_dt,
                                   kind="ExternalInput"))
        w2_d.append(nc.dram_tensor(f"w2{j}", [128, HTQ * D], io_dt,
                                   kind="ExternalInput"))
        y_d.append(nc.dram_tensor(f"y{j}", [128, DT * caps[j]], y_dt,
                                  kind="ExternalOutput"))
    b1_d = nc.dram_tensor("b1", [NJ * HQ], dt.float32, kind="ExternalInput")

    xt_v = [t.ap().rearrange("p (kt c) -> p kt c", kt=KT) for t in xt_d]
    w1_v = [t.ap().rearrange("p (ht kt h) -> p ht kt h", ht=HTQ, kt=KT)
            for t in w1_d]
    w2_v = [t.ap().rearrange("p (hh d) -> p hh d", hh=HTQ) for t in w2_d]
    y_v = [y_d[j].ap().rearrange("p (dt c) -> p dt c", dt=DT)
           for j in range(NJ)]
    b1_v = b1_d.ap().rearrange("(j ht p) -> p j ht", j=NJ, ht=HTQ)

    esz = 2
    cmax = max(caps)
    need = (
        2 * KT * cmax * esz         # xt double buffer
        + 2 * HTQ * KT * 128 * esz  # w1 double buffer
        + 2 * HTQ * 1024 * esz      # w2 double buffer
        + 2 * HTQ * cmax * esz      # h1q double buffer
        + 2 * DT * cmax * esz       # ybf staging double buffer
        + NJ * HTQ * 4 + 1024
    )
    assert need <= 190 * 1024, f"SBUF over budget: {need // 1024}KB"

    with tile.TileContext(nc) as tc:
        with (
            tc.tile_pool(name="xt", bufs=2) as xt_pool,
            tc.tile_pool(name="b1", bufs=1) as b1_pool,
            tc.tile_pool(name="ybfp", bufs=2) as ybf_pool,
            tc.tile_pool(name="w1q", bufs=2) as w1_pool,
            tc.tile_pool(name="w2q", bufs=2) as w2_pool,
            tc.tile_pool(name="h1q", bufs=2) as h1_pool,
            tc.tile_pool(name="ps1", bufs=3, space="PSUM") as ps1_pool,
            tc.tile_pool(name="ps2", bufs=4, space="PSUM") as ps2_pool,
            tc.tile_pool(name="warm", bufs=1) as warm_pool,
            tc.tile_pool(name="warmps", bufs=1, space="PSUM") as warmps_pool,
        ):
            # PE warm-up bridging the HBM fill phase
            wt = warm_pool.tile([128, 512], dt.bfloat16)
            nc.gpsimd.memset(wt[:], 0.0)
            wps = warmps_pool.tile([128, 512], dt.float32)
            for _ in range(16):
                nc.tensor.matmul(wps[:], wt[:, :128], wt[:], start=True, stop=True)

            # single sync-HWDGE queue, consumption-priority order:
            # w1(0)-ht0, xt(0), b1, w1(0)-ht1..7 | w2(0), w1(1), xt(1) |
            # then one job ahead inside the loop
            w1q_cur = w1_pool.tile([128, HTQ, KT, 128], io_dt)
            nc.sync.dma_start(w1q_cur[:, 0:1], w1_v[0][:, 0:1])

            cmax = max(caps)
            xt_cur = xt_pool.tile([128, KT, cmax], io_dt, name="xtt")
            ch0 = _chunks(caps[0])
            nc.sync.dma_start(
                xt_cur[:, :, : ch0[0][1]], xt_v[0][:, :, : ch0[0][1]]
            )
            nc.sync.dma_start(w1q_cur[:, 1:2], w1_v[0][:, 1:2])
            nc.sync.dma_start(w1q_cur[:, 2:3], w1_v[0][:, 2:3])
            b1t = b1_pool.tile([128, NJ, HTQ], dt.float32)
            nc.sync.dma_start(b1t[:], b1_v)
            for ht in range(3, HTQ):
                nc.sync.dma_start(
                    w1q_cur[:, ht : ht + 1], w1_v[0][:, ht : ht + 1]
                )
            for c0, n in ch0[1:]:
                nc.sync.dma_start(
                    xt_cur[:, :, c0 : c0 + n], xt_v[0][:, :, c0 : c0 + n]
                )

            def fetch_w1(j):
                w1q = w1_pool.tile([128, HTQ, KT, 128], io_dt)
                nc.sync.dma_start(w1q[:], w1_v[j][:])
                return w1q

            def fetch_w2(j):
                w2q = w2_pool.tile([128, HTQ, 1024], io_dt)
                nc.sync.dma_start(w2q[:], w2_v[j][:])
                return w2q

            def fetch_xt(j):
                xtn = xt_pool.tile([128, KT, cmax], io_dt, name="xtt")
                nc.sync.dma_start(xtn[:, :, : caps[j]], xt_v[j][:])
                return xtn

            w2q_cur = fetch_w2(0)
            w1q_nxt = fetch_w1(1)
            xt_nxt = fetch_xt(1)

            for j in range(NJ):
                if j >= 1:
                    w1q_cur, w1q_nxt = w1q_nxt, None
                    xt_cur, xt_nxt = xt_nxt, None
                    w2q_cur = fetch_w2(j)
                    if j + 1 < NJ:
                        w1q_nxt = fetch_w1(j + 1)
                        xt_nxt = fetch_xt(j + 1)
                w1q, w2q, xtj = w1q_cur, w2q_cur, xt_cur
                Cj = caps[j]
                chunks = _chunks(Cj)
                h1q = h1_pool.tile([128, HTQ, cmax], io_dt, name="h1t")
                ybf = ybf_pool.tile([128, DT, cmax], y_dt, name="ybft")

                # ---- layer 1: H1T[h, tok] = relu(W1qj^T @ XTj + b1) ----
                # job 0 runs chunk-major: phase [c0: ht0..7] consumes W1
                # at 145GB/s over 13.8us (vs all-of-xt in 3.4us), so the
                # PE starts on a 1.25MB prefix ~4us earlier
                l1_order = (
                    [(ht, cc) for cc in chunks for ht in range(HTQ)]
                    if j == 0 else
                    [(ht, cc) for ht in range(HTQ) for cc in chunks]
                )
                for ht, (c0, n) in l1_order:
                    ps = ps1_pool.tile([128, 512], dt.float32, tag="ps1")
                    for kt in range(KT):
                        nc.tensor.matmul(
                            ps[:, :n],
                            w1q[:, ht, kt],
                            xtj[:, kt, c0 : c0 + n],
                            start=(kt == 0),
                            stop=(kt == KT - 1),
                        )
                    nc.scalar.activation(
                        h1q[:, ht, c0 : c0 + n],
                        ps[:, :n],
                        mybir.ActivationFunctionType.Relu,
                        bias=b1t[:, j, ht : ht + 1],
                    )

                # ---- layer 2: YTj[d, tok] = W2qj^T @ H1T (no accum) ----
                for dtile in range(DT):
                    chunks2 = chunks
                    if j == NJ - 1 and dtile == DT - 1:
                        # split the very last chunk: its copy+store
                        # overlaps the closing matmuls (shorter tail)
                        c0l, nl = chunks[-1]
                        h1 = (nl // 2 + 3) & ~3
                        chunks2 = chunks[:-1] + [
                            (c0l, h1),
                            (c0l + h1, nl - h1),
                        ]
                    for c0, n in chunks2:
                        ps = ps2_pool.tile([128, 512], dt.float32, tag="ps2")
                        for ht in range(HTQ):
                            nc.tensor.matmul(
                                ps[:, :n],
                                w2q[:, ht, dtile * 128 : (dtile + 1) * 128],
                                h1q[:, ht, c0 : c0 + n],
                                start=(ht == 0),
                                stop=(ht == HTQ - 1),
                            )
                        yo = ybf[:, dtile, c0 : c0 + n]
                        nc.vector.tensor_copy(yo, ps[:, :n])
                        nc.scalar.dma_start(
                            y_v[j][:, dtile, c0 : c0 + n], yo
                        )

    nc.compile()
    return nc


def _get_nc(caps: tuple):
    key = (caps, MM_DT)
    if key not in _BUILD_CACHE:
        _BUILD_CACHE[key] = build_nc(caps)
    return _BUILD_CACHE[key]


def _pretile_w1q(w1q: np.ndarray, io_np) -> np.ndarray:
    # [D, HQ] -> [128, HTQ*KT*128]
    return np.ascontiguousarray(
        w1q.reshape(KT, 128, HTQ, 128)
        .transpose(1, 2, 0, 3)
        .reshape(128, HTQ * KT * 128)
        .astype(io_np, copy=False)
    )


def _pretile_w2q(w2q: np.ndarray, io_np) -> np.ndarray:
    # [HQ, D] -> [128, HTQ*D]
    return np.ascontiguousarray(
        w2q.reshape(HTQ, 128, D).transpose(1, 0, 2).reshape(128, HTQ * D)
        .astype(io_np, copy=False)
    )


def _assign_jobs(loads):
    """Greedy-balance 32 (expert, h-quarter) jobs onto NCORES cores of
    NJ slots each; returns per-core job lists + per-slot caps."""
    jobs = []  # (load, expert, quarter)
    for e, L in enumerate(loads):
        for qh in range(NJ):
            jobs.append((L, e, qh))
    jobs.sort(key=lambda t: -t[0])
    totals = [0] * NCORES
    core_jobs = [[] for _ in range(NCORES)]
    for L, e, qh in jobs:
        cands = [c for c in range(NCORES) if len(core_jobs[c]) < NJ]
        c = min(cands, key=lambda c: totals[c])
        core_jobs[c].append((L, e, qh))
        totals[c] += L
    # slot order: biggest job first on every core, so per-slot caps
    # (max over cores) stay tight
    for c in range(NCORES):
        core_jobs[c].sort(key=lambda t: -t[0])
    caps = tuple(
        min(1280, max(256, (max(core_jobs[c][s][0] for c in range(NCORES))
                            + 3) & ~3))
        for s in range(NJ)
    )
    return core_jobs, caps


def kernel(x, W1, b1, W2, b2, assign, k, _want_trace=False):
    x = np.asarray(x, dtype=np.float32)
    W1 = np.asarray(W1, dtype=np.float32)
    b1 = np.asarray(b1, dtype=np.float32)
    W2 = np.asarray(W2, dtype=np.float32)
    b2 = np.asarray(b2, dtype=np.float32)
    assign = np.asarray(assign)
    kk = int(k)

    assert W1.shape[0] == E and W2.shape[0] == E, "expert count must be 8"
    Bx, Sx, Dx = x.shape
    T = Bx * Sx
    xf = x.reshape(T, Dx)
    xT = np.ascontiguousarray(xf.T)  # [D, T]
    a2 = assign.reshape(T, -1)

    idx = [np.nonzero((a2 == e).any(axis=1))[0] for e in range(E)]
    loads = [len(i) for i in idx]
    if max(loads) > 1280:
        # cannot happen for the spec'd uniform-random assignment sizes
        raise ValueError("expert load exceeds device capacity")

    core_jobs, caps = _assign_jobs(loads)
    nc = _get_nc(caps)
    io_np = _io_np_dtype()

    # per-expert pre-tiled xt (shared by its 4 quarter-jobs)
    xt_pre = {}
    for e in range(E):
        n = loads[e]
        if n:
            xt_pre[e] = np.ascontiguousarray(
                xT[:, idx[e]].reshape(KT, 128, n).transpose(1, 0, 2)
                .astype(io_np, copy=False)
            )

    in_maps = []
    for c in range(NCORES):
        m = {}
        b1c = np.zeros(NJ * HQ, dtype=np.float32)
        for s in range(NJ):
            L, e, qh = core_jobs[c][s]
            Cj = caps[s]
            xt_buf = np.zeros((128, KT, Cj), dtype=io_np)
            if L:
                xt_buf[:, :, :L] = xt_pre[e]
            m[f"xt{s}"] = xt_buf.reshape(128, KT * Cj)
            m[f"w1{s}"] = _pretile_w1q(
                W1[e][:, qh * HQ : (qh + 1) * HQ], io_np)
            m[f"w2{s}"] = _pretile_w2q(
                W2[e][qh * HQ : (qh + 1) * HQ, :], io_np)
            b1c[s * HQ : (s + 1) * HQ] = b1[e][qh * HQ : (qh + 1) * HQ]
        m["b1"] = b1c
        in_maps.append(m)

    res = run_bass_kernel_spmd(
        nc,
        in_maps,
        core_ids=list(range(NCORES)),
        trace=_want_trace,
        trace_cores=list(range(NCORES)) if _want_trace else None,
    )

    out_f = np.zeros((T, Dx), dtype=np.float32)
    for c in range(NCORES):
        for s in range(NJ):
            L, e, qh = core_jobs[c][s]
            if not L:
                continue
            Cj = caps[s]
            yt = (
                res.results[c][f"y{s}"]
                .reshape(128, DT, Cj)
                .transpose(1, 0, 2)
                .reshape(Dx, Cj)[:, :L]
            )
            out_f[idx[e]] += yt.T.astype(np.float32)
    for e in range(E):
        if loads[e]:
            out_f[idx[e]] += b2[e][None, :]

    out = (out_f * np.float32(1.0 / kk)).reshape(Bx, Sx, Dx)
    if _want_trace:
        return out, res
    return out


# revision 43
# speedup vs baseline: 1.0196x; 1.0196x over previous
"""MoE layer (hash-routed, top-k=2, E=8 experts) on 8 Trainium2 NeuronCores.

Strategy: expert-parallel with QUARTER-LEVEL load balancing.  Each expert's
H=4096 hidden dim splits into 4 quarters of 1024; since ReLU is elementwise
in h, y_e = sum_j relu(x@W1[:, Qj] + b1[Qj]) @ W2[Qj, :] exactly.  That
gives 32 independent (expert, h-quarter) jobs, greedily packed 4-per-core
so every core's token total is ~mean expert load instead of the hottest
expert's load.  The host routes tokens, pre-tiles weights partition-major,
and sums the 4 partial outputs per expert (plus b2) during unscatter.

All matmul operands are bf16 (same 1-cycle/row PE rate as f32r, half the
traffic, LDWEIGHTS fully hidden); PSUM accumulates fp32; the partial y of
a job needs no cross-quarter SBUF accumulation, so layer 2 results go
PSUM -> bf16 -> HBM directly.

Per-core device kernel, jobs j=0..3 with per-slot token caps C_j:
  job j: layer1: H1T[h, tok] = relu(W1qj^T @ XTj + b1qj)
         layer2: YTj[d, tok] = W2qj^T @ H1T          (stored per job)
All loads ride one sync-HWDGE queue in consumption-priority order; 34
warm-up matmuls bridge the ~8us HBM fill phase so the PE clock ramp is
never reset by an idle gap.  Tokens are the moving/free dim in both
layers, so caps need no 128 padding.
"""

import numpy as np

import concourse.bass as bass
import concourse.mybir as mybir
import concourse.tile as tile
from concourse import bacc
from concourse.bass_utils import run_bass_kernel_spmd

dt = mybir.dt

B, S, D, H, E, NCORES = 4, 1024, 1024, 4096, 8, 8
HQ = 1024                      # h-quarter width
KT = D // 128                  # 8 contraction tiles (d)
DT = D // 128                  # 8 output d-tiles
HTQ = HQ // 128                # 8 h-tiles per quarter
NJ = H // HQ                   # 4 jobs per core (one h-quarter each)

MM_DT = "bf16"

_BUILD_CACHE: dict = {}


def _io_np_dtype():
    if MM_DT == "bf16":
        import ml_dtypes

        return np.dtype(ml_dtypes.bfloat16)
    return np.dtype(np.float32)


def _io_dt():
    return dt.bfloat16 if MM_DT == "bf16" else dt.float32


def _chunks(C):
    """Token chunks of <=512 (PSUM bank limit for fp32 out)."""
    out = []
    c0 = 0
    while c0 < C:
        n = min(512, C - c0)
        out.append((c0, n))
        c0 += n
    return out


def build_nc(caps: tuple):
    """Per-core program: NJ jobs with token caps caps[j]."""
    assert len(caps) == NJ
    for c in caps:
        assert c % 4 == 0 and 256 <= c <= 1280
    io_dt = _io_dt()
    y_dt = dt.bfloat16 if MM_DT == "bf16" else dt.float32

    nc = bacc.Bacc(
        "TRN2",
        target_bir_lowering=False,
        debug=False,
        num_devices=NCORES,
    )

    # per-job tensors, host pre-tiled partition-major:
    #   xtJ: [128, KT*Cj]     xt[p, kt*Cj+c] = x_tok[c, kt*128+p]
    #   w1J: [128, HTQ*KT*128] quarter slice of W1, tiled like xt
    #   w2J: [128, HTQ*D]      quarter rows of W2
    #   yJ:  [128, DT*Cj]      YT[d, tok] partial (bf16)
    xt_d, w1_d, w2_d, y_d = [], [], [], []
    for j in range(NJ):
        xt_d.append(nc.dram_tensor(f"xt{j}", [128, KT * caps[j]], io_dt,
                                   kind="ExternalInput"))
        w1_d.append(nc.dram_tensor(f"w1{j}", [128, HTQ * KT * 128], io_dt,
                                   kind="ExternalInput"))
        w2_d.append(nc.dram_tensor(f"w2{j}", [128, HTQ * D], io_dt,
                                   kind="ExternalInput"))
        y_d.append(nc.dram_tensor(f"y{j}", [128, DT * caps[j]], y_dt,
                                  kind="ExternalOutput"))
    b1_d = nc.dram_tensor("b1", [NJ * HQ], dt.float32, kind="ExternalInput")

    xt_v = [t.ap().rearrange("p (kt c) -> p kt c", kt=KT) for t in xt_d]
    w1_v = [t.ap().rearrange("p (ht kt h) -> p ht kt h", ht=HTQ, kt=KT)
            for t in w1_d]
    w2_v = [t.ap().rearrange("p (hh d) -> p hh d", hh=HTQ) for t in w2_d]
    y_v = [y_d[j].ap().rearrange("p (dt c) -> p dt c", dt=DT)
           for j in range(NJ)]
    b1_v = b1_d.ap().rearrange("(j ht p) -> p j ht", j=NJ, ht=HTQ)

    esz = 2
    cmax = max(caps)
    need = (
        2 * KT * cmax * esz         # xt double buffer
        + 2 * HTQ * KT * 128 * esz  # w1 double buffer
        + 2 * HTQ * 1024 * esz      # w2 double buffer
        + 2 * HTQ * cmax * esz      # h1q double buffer
        + 2 * DT * cmax * esz       # ybf staging double buffer
        + NJ * HTQ * 4 + 1024
    )
    assert need <= 190 * 1024, f"SBUF over budget: {need // 1024}KB"

    with tile.TileContext(nc) as tc:
        with (
            tc.tile_pool(name="xt", bufs=2) as xt_pool,
            tc.tile_pool(name="b1", bufs=1) as b1_pool,
            tc.tile_pool(name="ybfp", bufs=2) as ybf_pool,
            tc.tile_pool(name="w1q", bufs=2) as w1_pool,
            tc.tile_pool(name="w2q", bufs=2) as w2_pool,
            tc.tile_pool(name="h1q", bufs=2) as h1_pool,
            tc.tile_pool(name="ps1", bufs=3, space="PSUM") as ps1_pool,
            tc.tile_pool(name="ps2", bufs=4, space="PSUM") as ps2_pool,
            tc.tile_pool(name="warm", bufs=1) as warm_pool,
            tc.tile_pool(name="warmps", bufs=1, space="PSUM") as warmps_pool,
        ):
            # PE warm-up bridging the HBM fill phase
            wt = warm_pool.tile([128, 512], dt.bfloat16)
            nc.gpsimd.memset(wt[:], 0.0)
            wps = warmps_pool.tile([128, 512], dt.float32)
            for _ in range(22):
                nc.tensor.matmul(wps[:], wt[:, :128], wt[:], start=True, stop=True)

            # single sync-HWDGE queue, consumption-priority order:
            # w1(0)-ht0, xt(0), b1, w1(0)-ht1..7 | w2(0), w1(1), xt(1) |
            # then one job ahead inside the loop
            w1q_cur = w1_pool.tile([128, HTQ, KT, 128], io_dt)
            nc.sync.dma_start(w1q_cur[:, 0:1], w1_v[0][:, 0:1])

            cmax = max(caps)
            xt_cur = xt_pool.tile([128, KT, cmax], io_dt, name="xtt")
            ch0 = _chunks(caps[0])
            nc.sync.dma_start(
                xt_cur[:, :, : ch0[0][1]], xt_v[0][:, :, : ch0[0][1]]
            )
            nc.sync.dma_start(w1q_cur[:, 1:2], w1_v[0][:, 1:2])
            nc.sync.dma_start(w1q_cur[:, 2:3], w1_v[0][:, 2:3])
            b1t = b1_pool.tile([128, NJ, HTQ], dt.float32)
            nc.sync.dma_start(b1t[:], b1_v)
            for ht in range(3, HTQ):
                nc.sync.dma_start(
                    w1q_cur[:, ht : ht + 1], w1_v[0][:, ht : ht + 1]
                )
            for c0, n in ch0[1:]:
                nc.sync.dma_start(
                    xt_cur[:, :, c0 : c0 + n], xt_v[0][:, :, c0 : c0 + n]
                )

            def fetch_w1(j):
                w1q = w1_pool.tile([128, HTQ, KT, 128], io_dt)
                nc.sync.dma_start(w1q[:], w1_v[j][:])
                return w1q

            def fetch_w2(j):
                w2q = w2_pool.tile([128, HTQ, 1024], io_dt)
                nc.sync.dma_start(w2q[:], w2_v[j][:])
                return w2q

            def fetch_xt(j):
                xtn = xt_pool.tile([128, KT, cmax], io_dt, name="xtt")
                nc.sync.dma_start(xtn[:, :, : caps[j]], xt_v[j][:])
                return xtn

            w2q_cur = fetch_w2(0)
            w1q_nxt = fetch_w1(1)
            xt_nxt = fetch_xt(1)

            for j in range(NJ):
                if j >= 1:
                    w1q_cur, w1q_nxt = w1q_nxt, None
                    xt_cur, xt_nxt = xt_nxt, None
                    w2q_cur = fetch_w2(j)
                    if j + 1 < NJ:
                        w1q_nxt = fetch_w1(j + 1)
                        xt_nxt = fetch_xt(j + 1)
                w1q, w2q, xtj = w1q_cur, w2q_cur, xt_cur
                Cj = caps[j]
                chunks = _chunks(Cj)
                h1q = h1_pool.tile([128, HTQ, cmax], io_dt, name="h1t")
                ybf = ybf_pool.tile([128, DT, cmax], y_dt, name="ybft")

                # ---- layer 1: H1T[h, tok] = relu(W1qj^T @ XTj + b1) ----
                # job 0 runs chunk-major: phase [c0: ht0..7] consumes W1
                # at 145GB/s over 13.8us (vs all-of-xt in 3.4us), so the
                # PE starts on a 1.25MB prefix ~4us earlier
                l1_order = (
                    [(ht, cc) for cc in chunks for ht in range(HTQ)]
                    if j == 0 else
                    [(ht, cc) for ht in range(HTQ) for cc in chunks]
                )
                for ht, (c0, n) in l1_order:
                    ps = ps1_pool.tile([128, 512], dt.float32, tag="ps1")
                    for kt in range(KT):
                        nc.tensor.matmul(
                            ps[:, :n],
                            w1q[:, ht, kt],
                            xtj[:, kt, c0 : c0 + n],
                            start=(kt == 0),
                            stop=(kt == KT - 1),
                        )
                    nc.scalar.activation(
                        h1q[:, ht, c0 : c0 + n],
                        ps[:, :n],
                        mybir.ActivationFunctionType.Relu,
                        bias=b1t[:, j, ht : ht + 1],
                    )

                # ---- layer 2: YTj[d, tok] = W2qj^T @ H1T (no accum) ----
                for dtile in range(DT):
                    chunks2 = chunks
                    if j == NJ - 1 and dtile == DT - 1:
                        # split the very last chunk: its copy+store
                        # overlaps the closing matmuls (shorter tail)
                        c0l, nl = chunks[-1]
                        h1 = (nl // 2 + 3) & ~3
                        chunks2 = chunks[:-1] + [
                            (c0l, h1),
                            (c0l + h1, nl - h1),
                        ]
                    for c0, n in chunks2:
                        ps = ps2_pool.tile([128, 512], dt.float32, tag="ps2")
                        for ht in range(HTQ):
                            nc.tensor.matmul(
                                ps[:, :n],
                                w2q[:, ht, dtile * 128 : (dtile + 1) * 128],
                                h1q[:, ht, c0 : c0 + n],
                                start=(ht == 0),
                                stop=(ht == HTQ - 1),
                            )
                        yo = ybf[:, dtile, c0 : c0 + n]
                        nc.vector.tensor_copy(yo, ps[:, :n])
                        nc.scalar.dma_start(
                            y_v[j][:, dtile, c0 : c0 + n], yo
                        )

    nc.compile()
    return nc


def _get_nc(caps: tuple):
    key = (caps, MM_DT)
    if key not in _BUILD_CACHE:
        _BUILD_CACHE[key] = build_nc(caps)
    return _BUILD_CACHE[key]


def _pretile_w1q(w1q: np.ndarray, io_np) -> np.ndarray:
    # [D, HQ] -> [128, HTQ*KT*128]
    return np.ascontiguousarray(
        w1q.reshape(KT, 128, HTQ, 128)
        .transpose(1, 2, 0, 3)
        .reshape(128, HTQ * KT * 128)
        .astype(io_np, copy=False)
    )


def _pretile_w2q(w2q: np.ndarray, io_np) -> np.ndarray:
    # [HQ, D] -> [128, HTQ*D]
    return np.ascontiguousarray(
        w2q.reshape(HTQ, 128, D).transpose(1, 0, 2).reshape(128, HTQ * D)
        .astype(io_np, copy=False)
    )


def _assign_jobs(loads):
    """Greedy-balance 32 (expert, h-quarter) jobs onto NCORES cores of
    NJ slots each; returns per-core job lists + per-slot caps."""
    jobs = []  # (load, expert, quarter)
    for e, L in enumerate(loads):
        for qh in range(NJ):
            jobs.append((L, e, qh))
    jobs.sort(key=lambda t: -t[0])
    totals = [0] * NCORES
    core_jobs = [[] for _ in range(NCORES)]
    for L, e, qh in jobs:
        cands = [c for c in range(NCORES) if len(core_jobs[c]) < NJ]
        c = min(cands, key=lambda c: totals[c])
        core_jobs[c].append((L, e, qh))
        totals[c] += L
    # slot order: biggest job first on every core, so per-slot caps
    # (max over cores) stay tight
    for c in range(NCORES):
        core_jobs[c].sort(key=lambda t: -t[0])
    caps = tuple(
        min(1280, max(256, (max(core_jobs[c][s][0] for c in range(NCORES))
                            + 3) & ~3))
        for s in range(NJ)
    )
    return core_jobs, caps


def kernel(x, W1, b1, W2, b2, assign, k, _want_trace=False):
    x = np.asarray(x, dtype=np.float32)
    W1 = np.asarray(W1, dtype=np.float32)
    b1 = np.asarray(b1, dtype=np.float32)
    W2 = np.asarray(W2, dtype=np.float32)
    b2 = np.asarray(b2, dtype=np.float32)
    assign = np.asarray(assign)
    kk = int(k)

    assert W1.shape[0] == E and W2.shape[0] == E, "expert count must be 8"
    Bx, Sx, Dx = x.shape
    T = Bx * Sx
    xf = x.reshape(T, Dx)
    xT = np.ascontiguousarray(xf.T)  # [D, T]
    a2 = assign.reshape(T, -1)

    idx = [np.nonzero((a2 == e).any(axis=1))[0] for e in range(E)]
    loads = [len(i) for i in idx]
    if max(loads) > 1280:
        # cannot happen for the spec'd uniform-random assignment sizes
        raise ValueError("expert load exceeds device capacity")

    core_jobs, caps = _assign_jobs(loads)
    nc = _get_nc(caps)
    io_np = _io_np_dtype()

    # per-expert pre-tiled xt (shared by its 4 quarter-jobs)
    xt_pre = {}
    for e in range(E):
        n = loads[e]
        if n:
            xt_pre[e] = np.ascontiguousarray(
                xT[:, idx[e]].reshape(KT, 128, n).transpose(1, 0, 2)
                .astype(io_np, copy=False)
            )

    in_maps = []
    for c in range(NCORES):
        m = {}
        b1c = np.zeros(NJ * HQ, dtype=np.float32)
        for s in range(NJ):
            L, e, qh = core_jobs[c][s]
            Cj = caps[s]
            xt_buf = np.zeros((128, KT, Cj), dtype=io_np)
            if L:
                xt_buf[:, :, :L] = xt_pre[e]
            m[f"xt{s}"] = xt_buf.reshape(128, KT * Cj)
            m[f"w1{s}"] = _pretile_w1q(
                W1[e][:, qh * HQ : (qh + 1) * HQ], io_np)
            m[f"w2{s}"] = _pretile_w2q(
                W2[e][qh * HQ : (qh + 1) * HQ, :], io_np)
            b1c[s * HQ : (s + 1) * HQ] = b1[e][qh * HQ : (qh + 1) * HQ]
        m["b1"] = b1c
        in_maps.append(m)

    res = run_bass_kernel_spmd(
        nc,
        in_maps,
        core_ids=list(range(NCORES)),
        trace=_want_trace,
        trace_cores=list(range(NCORES)) if _want_trace else None,
    )

    out_f = np.zeros((T, Dx), dtype=np.float32)
    for c in range(NCORES):
        for s in range(NJ):
            L, e, qh = core_jobs[c][s]
            if not L:
                continue
            Cj = caps[s]
            yt = (
                res.results[c][f"y{s}"]
                .reshape(128, DT, Cj)
                .transpose(1, 0, 2)
                .reshape(Dx, Cj)[:, :L]
            )
            out_f[idx[e]] += yt.T.astype(np.float32)
    for e in range(E):
        if loads[e]:
            out_f[idx[e]] += b2[e][None, :]

    out = (out_f * np.float32(1.0 / kk)).reshape(Bx, Sx, Dx)
    if _want_trace:
        return out, res
    return out
